# revision 1
# baseline (speedup 1.0000x reference)
"""Sparse (graph-masked) multi-head attention on 8 Trainium2 NeuronCores.

Reference computation (fp32, single device):
    qkv = x @ w_qkv + b_qkv ; split heads (H=8, D=64)
    scores = q k^T / sqrt(D), masked by adj_matrix (True=attend)
    y = softmax(scores) @ v ; out = y @ w_proj + b_proj

Sharding: core = (batch b, query-half th).  Each core owns queries
t in [th*1024, (th+1)*1024) of batch b and produces out[b, that slice, :].
No cross-core communication.

Device layout (per core), everything bf16 except PSUM accum + final out:
    xT      [C, T]   = x[b].T            (keys/values need full T)
    xTq     [C, TL]  = x[b].T local-t slice (queries)
    maskT   [T, TL]  = adj[b].T slice, as bf16 0/1
    qT,kT   [C, *]   via matmul  (c-major so heads are partition slices)
    v_aug   [T, H*65] v with a ones column per head (65th col) so that
            attnT.T@v_aug yields both y and the softmax denominator
    scoresT [s,t] per head in PSUM -> ACT exp (scale=1/8) -> bf16 attnT
            -> DVE multiply by maskT (masked entries contribute 0)
    psum_y  [65, TL] accumulates over the 16 s-chunks
    yT_h    [64, TL] = psum_y[0:64] * broadcast(1/psum_y[64])
    out     [TL, C]  = sum_h yT_h.T @ w_proj[h-slice] + b_proj (fp32)

Softmax max-subtraction is skipped: scores*scale ~ N(0, 0.2^2) here, so
exp never overflows and masked entries are exactly zeroed by the mask
multiply (denominator >= exp(self-edge) > 0).
"""

import numpy as np
import ml_dtypes

import concourse.bass as bass
import concourse.mybir as mybir
import concourse.tile as tile
from concourse import bacc
from concourse.bass_utils import run_bass_kernel_spmd

BF16 = mybir.dt.bfloat16
F32 = mybir.dt.float32
nbf16 = ml_dtypes.bfloat16

B, T, C, H = 4, 2048, 512, 8
D = C // H          # 64
P = 128
NCORES = 8
TL = T // 2         # queries per core
SCALE = 1.0 / float(np.sqrt(D))

AF = mybir.ActivationFunctionType
ALU = mybir.AluOpType


def build_program(t_full=T, t_local=TL, loop_reps=1, num_devices=NCORES,
                  probe=None):
    """Build the SPMD Bass program (identical on all cores)."""
    nkc = C // P                # contraction chunks over C
    nsc = t_full // P           # key/s chunks
    ntc = t_local // P          # output t chunks
    VW = D + 1                  # v columns per head incl. ones column

    nc = bacc.Bacc("TRN2", target_bir_lowering=False, debug=False,
                   num_devices=num_devices)

    xT = nc.dram_tensor("xT", [C, t_full], BF16, kind="ExternalInput").ap()
    xTq = nc.dram_tensor("xTq", [C, t_local], BF16, kind="ExternalInput").ap()
    maskT = nc.dram_tensor("maskT", [t_full, t_local], BF16,
                           kind="ExternalInput").ap()
    wq = nc.dram_tensor("wq", [C, C], BF16, kind="ExternalInput").ap()
    wk = nc.dram_tensor("wk", [C, C], BF16, kind="ExternalInput").ap()
    wv = nc.dram_tensor("wv", [C, C], BF16, kind="ExternalInput").ap()
    wp = nc.dram_tensor("wp", [C, C], BF16, kind="ExternalInput").ap()
    bq = nc.dram_tensor("bq", [C], F32, kind="ExternalInput").ap()
    bk = nc.dram_tensor("bk", [C], F32, kind="ExternalInput").ap()
    bv = nc.dram_tensor("bv", [1, C], F32, kind="ExternalInput").ap()
    bp = nc.dram_tensor("bp", [1, C], F32, kind="ExternalInput").ap()
    out = nc.dram_tensor("out", [t_local, C], F32, kind="ExternalOutput").ap()

    with tile.TileContext(nc) as tc:
        def body():
            with tc.tile_pool(name="persist", bufs=1) as pp:
                # ---- loads ----
                dma_reps = 2 if probe == "dma" else 1
                xT_sb = pp.tile([P, nkc, t_full], BF16, tag="xT")
                for _ in range(dma_reps):
                    nc.sync.dma_start(
                        xT_sb[:], xT.rearrange("(k p) t -> p k t", p=P))
                xTq_sb = pp.tile([P, nkc, t_local], BF16, tag="xTq")
                nc.sync.dma_start(
                    xTq_sb[:], xTq.rearrange("(k p) t -> p k t", p=P))
                # mask chunks on the (otherwise idle) Pool sequencer, in 4
                # groups so attention can start before the whole mask lands
                mask_sb = pp.tile([P, nsc, t_local], BF16, tag="mask")
                mask_r = maskT.rearrange("(i p) t -> p i t", p=P)
                ngrp = min(4, nsc)
                for _ in range(dma_reps):
                    for g in range(ngrp):
                        gs = nsc // ngrp
                        nc.gpsimd.dma_start(mask_sb[:, g * gs:(g + 1) * gs],
                                            mask_r[:, g * gs:(g + 1) * gs])
                w_sb = {}
                for name, w in (("wq", wq), ("wk", wk), ("wv", wv), ("wp", wp)):
                    w_sb[name] = pp.tile([P, nkc, C], BF16, tag=name, name=name)
                    nc.sync.dma_start(
                        w_sb[name][:], w.rearrange("(k p) c -> p k c", p=P))
                # per-partition bias columns: bq_sb[:, j] = bq[128j : 128j+128]
                bq_sb = pp.tile([P, nkc], F32, tag="bq")
                nc.sync.dma_start(bq_sb[:], bq.rearrange("(j p) -> p j", p=P))
                bk_sb = pp.tile([P, nkc], F32, tag="bk")
                nc.sync.dma_start(bk_sb[:], bk.rearrange("(j p) -> p j", p=P))
                # broadcast-along-partition biases (free-dim indexed)
                bv_row = pp.tile([1, C], F32, tag="bv_row")
                nc.sync.dma_start(bv_row[:], bv[:])
                bp_row = pp.tile([1, C], F32, tag="bp_row")
                nc.sync.dma_start(bp_row[:], bp[:])
                bv_bc = pp.tile([P, C], F32, tag="bv_bc")
                nc.gpsimd.partition_broadcast(bv_bc[:], bv_row[:])
                bp_bc = pp.tile([P, C], F32, tag="bp_bc")
                nc.gpsimd.partition_broadcast(bp_bc[:], bp_row[:])

                qT_sb = pp.tile([P, nkc, t_local], BF16, tag="qT")
                kT_sb = pp.tile([P, nkc, t_full], BF16, tag="kT")
                v_sb = pp.tile([P, nsc, H * VW], BF16, tag="v")
                yT_sb = [pp.tile([D, t_local], BF16, tag=f"yT{h}",
                                 name=f"yT{h}")
                         for h in range(H)]
                # head pairs packed [128, t] for K=128 projection matmuls
                yT_pair = [pp.tile([P, t_local], BF16, tag=f"yTp{j}",
                                   name=f"yTp{j}")
                           for j in range(H // 2)]

                NB = 512     # max matmul output free size (one PSUM bank)

                def nslices(total):
                    return [slice(n, min(n + NB, total))
                            for n in range(0, total, NB)]

                # ---- phase 1: projections ----
                # emission order: q0,k0 first, then v, then remaining q/k
                # chunks, so attention pair 0 can start as early as possible.
                with tc.tile_pool(name="psum1", bufs=2, space="PSUM") as ps1:
                    p1_reps = 2 if probe == "phase1" else 1

                    def emit_q(j):
                        pq = ps1.tile([P, t_local], F32, tag="p1", name="pq")
                        for r in range(p1_reps):
                            for k in range(nkc):
                                for sl in nslices(t_local):
                                    nc.tensor.matmul(
                                        pq[:, sl],
                                        w_sb["wq"][:, k, j * P:(j + 1) * P],
                                        xTq_sb[:, k, sl],
                                        start=(k == 0), stop=(k == nkc - 1))
                        nc.vector.tensor_scalar_add(
                            qT_sb[:, j], pq[:], bq_sb[:, j:j + 1])

                    def emit_k(j):
                        pk = ps1.tile([P, t_full], F32, tag="p1", name="pk")
                        for r in range(p1_reps):
                            for k in range(nkc):
                                for sl in nslices(t_full):
                                    nc.tensor.matmul(
                                        pk[:, sl],
                                        w_sb["wk"][:, k, j * P:(j + 1) * P],
                                        xT_sb[:, k, sl],
                                        start=(k == 0), stop=(k == nkc - 1))
                        nc.vector.tensor_scalar_add(
                            kT_sb[:, j], pk[:], bk_sb[:, j:j + 1])

                    emit_q(0)
                    emit_k(0)
                    for i in range(nsc):       # v (t-major) + ones columns
                        pv = ps1.tile([P, C], F32, tag="p1", name="pv")
                        for k in range(nkc):
                            nc.tensor.matmul(
                                pv[:], xT_sb[:, k, i * P:(i + 1) * P],
                                w_sb["wv"][:, k], start=(k == 0),
                                stop=(k == nkc - 1))
                        nc.vector.memset(v_sb[:, i], 1.0)
                        v_dst = v_sb[:, i].rearrange(
                            "p (h w) -> p h w", w=VW)[:, :, 0:D]
                        nc.vector.scalar_tensor_tensor(
                            v_dst, pv[:].rearrange("p (h d) -> p h d", d=D),
                            0.0, bv_bc[:].rearrange("p (h d) -> p h d", d=D),
                            op0=ALU.add, op1=ALU.add)
                    for j in range(1, nkc):
                        emit_q(j)
                        emit_k(j)

                # ---- phase 2: attention, dual-chain interleaved ----
                # Two head-pairs (chains) alternate chunks: while chain A
                # waits its exp->mask->y latency, ACT/DVE/PE work on chain
                # B, so ACT stays saturated.  Heads 2p/2p+1 of each pair
                # use array row groups 0/64 (concurrent matmuls).
                # PSUM: psA 2 slots x 2 banks + 4 y-accumulators x 1 bank.
                TB = min(NB, t_local)    # t-block per pass (PSUM budget)
                with (tc.tile_pool(name="psA", bufs=2, space="PSUM") as psA,
                      tc.tile_pool(name="psY", bufs=1, space="PSUM") as psY,
                      tc.tile_pool(name="attn", bufs=6) as ap_pool,
                      tc.tile_pool(name="small", bufs=4) as sm_pool):
                    for pg in range((H // 2 + 1) // 2):
                        pairs = [q for q in (2 * pg, 2 * pg + 1)
                                 if q < H // 2]
                        for tb in range(t_local // TB):
                            tsl = slice(tb * TB, (tb + 1) * TB)
                            py = {}
                            for ci, p in enumerate(pairs):
                                py[ci, 0] = psY.tile([D + 1, TB], F32,
                                                     tag=f"y{ci}0",
                                                     name=f"py{ci}0")
                                py[ci, 1] = psY.tile([D + 1, TB], F32,
                                                     tag=f"y{ci}1",
                                                     name=f"py{ci}1")
                            ps_tiles = {}

                            def emit_scores(ci, i):
                                p = pairs[ci]
                                ps = psA.tile([P, 2 * TB], F32, tag="s",
                                              name="ps")
                                ps_tiles[ci, i] = ps
                                nc.tensor.matmul(
                                    ps[:, 0:TB],
                                    kT_sb[0:D, p, i * P:(i + 1) * P],
                                    qT_sb[0:D, p, tsl],
                                    start=True, stop=True,
                                    tile_position=(0, 0))
                                nc.tensor.matmul(
                                    ps[:, TB:2 * TB],
                                    kT_sb[D:P, p, i * P:(i + 1) * P],
                                    qT_sb[D:P, p, tsl],
                                    start=True, stop=True,
                                    tile_position=(D, 0))

                            for ci in range(len(pairs)):
                                emit_scores(ci, 0)
                            for i in range(nsc):
                                for ci, p in enumerate(pairs):
                                    h0, h1 = 2 * p, 2 * p + 1
                                    ps = ps_tiles.pop((ci, i))
                                    at = ap_pool.tile([P, 2 * TB], BF16,
                                                      tag="at")
                                    nc.scalar.activation(at[:], ps[:], AF.Exp,
                                                         scale=SCALE)
                                    mask_bc = mask_sb[:, i, tsl].rearrange(
                                        "p (o n) -> p o n",
                                        o=1).broadcast_to([P, 2, TB])
                                    am = ap_pool.tile([P, 2 * TB], BF16,
                                                      tag="am", name="am")
                                    nc.vector.tensor_mul(
                                        am[:].rearrange("p (g n) -> p g n",
                                                        g=2),
                                        at[:].rearrange("p (g n) -> p g n",
                                                        g=2),
                                        mask_bc)
                                    nc.tensor.matmul(
                                        py[ci, 0][:], v_sb[:, i].rearrange(
                                            "p (g w) -> p g w", w=VW)[:, h0],
                                        am[:, 0:TB], start=(i == 0),
                                        stop=(i == nsc - 1))
                                    nc.tensor.matmul(
                                        py[ci, 1][:], v_sb[:, i].rearrange(
                                            "p (g w) -> p g w", w=VW)[:, h1],
                                        am[:, TB:2 * TB], start=(i == 0),
                                        stop=(i == nsc - 1))
                                    # next chunk of this chain: its psA slot
                                    # frees once the exp above has read ps
                                    if i + 1 < nsc:
                                        emit_scores(ci, i + 1)
                            for ci, p in enumerate(pairs):
                                for j, h in ((0, 2 * p), (1, 2 * p + 1)):
                                    pyt = py[ci, j]
                                    recip = sm_pool.tile([1, TB], F32,
                                                         tag="recip")
                                    nc.vector.reciprocal(recip[:],
                                                         pyt[D:D + 1, :])
                                    rbc = sm_pool.tile([D, TB], F32,
                                                      tag="rbc")
                                    nc.gpsimd.partition_broadcast(rbc[:],
                                                                  recip[:])
                                    nc.vector.scalar_tensor_tensor(
                                        yT_sb[h][:, tsl], pyt[0:D, :], 0.0,
                                        rbc[:], op0=ALU.add, op1=ALU.mult)
                                    nc.gpsimd.dma_start(
                                        yT_pair[p][j * D:(j + 1) * D, tsl],
                                        yT_sb[h][:, tsl])

                # ---- phase 3: output projection ----
                with (tc.tile_pool(name="psO", bufs=2, space="PSUM") as psO,
                      tc.tile_pool(name="osb", bufs=2) as o_pool):
                    for tch in range(ntc):
                        po = psO.tile([P, C], F32, tag="o")
                        for j in range(H // 2):
                            nc.tensor.matmul(
                                po[:], yT_pair[j][:, tch * P:(tch + 1) * P],
                                w_sb["wp"][:, j],
                                start=(j == 0), stop=(j == H // 2 - 1))
                        o_sb = o_pool.tile([P, C], F32, tag="o_sb")
                        nc.vector.scalar_tensor_tensor(
                            o_sb[:], po[:], 0.0, bp_bc[:],
                            op0=ALU.add, op1=ALU.add)
                        nc.sync.dma_start(out[tch * P:(tch + 1) * P, :], o_sb[:])

        if loop_reps > 1:
            ET = mybir.EngineType
            with tc.For_i(0, loop_reps, 1,
                          hint_engines=(ET.PE, ET.DVE, ET.Activation,
                                        ET.Pool, ET.SP)):
                body()
        else:
            body()

    nc.compile()
    return nc


def shard_inputs(x, adj_matrix, w_qkv, b_qkv, w_proj, b_proj,
                 t_full=T, t_local=TL):
    """Host-side shard/layout prep. Core c handles (b, th) = divmod(c, 2)."""
    wq = np.ascontiguousarray(w_qkv[:, 0:C]).astype(nbf16)
    wk = np.ascontiguousarray(w_qkv[:, C:2 * C]).astype(nbf16)
    wv = np.ascontiguousarray(w_qkv[:, 2 * C:3 * C]).astype(nbf16)
    wp = np.ascontiguousarray(w_proj).astype(nbf16)
    bq = np.ascontiguousarray(b_qkv[0:C]).astype(np.float32)
    bk = np.ascontiguousarray(b_qkv[C:2 * C]).astype(np.float32)
    bv = np.ascontiguousarray(b_qkv[2 * C:3 * C]).astype(np.float32)[None]
    bp = np.ascontiguousarray(b_proj).astype(np.float32)[None]
    in_maps = []
    n_th = t_full // t_local
    for core in range(B * n_th):
        b, th = divmod(core, n_th)
        xTb = np.ascontiguousarray(x[b, :t_full].T).astype(nbf16)
        tsl = slice(th * t_local, (th + 1) * t_local)
        in_maps.append({
            "xT": xTb,
            "xTq": np.ascontiguousarray(xTb[:, tsl]),
            "maskT": np.ascontiguousarray(
                adj_matrix[b, :t_full, :t_full].T[:, tsl]).astype(nbf16),
            "wq": wq, "wk": wk, "wv": wv, "wp": wp,
            "bq": bq, "bk": bk, "bv": bv, "bp": bp,
        })
    return in_maps


_PROGRAM_CACHE = {}


def _get_program(key=(T, TL, 1)):
    if key not in _PROGRAM_CACHE:
        probe = key[3] if len(key) > 3 else None
        _PROGRAM_CACHE[key] = build_program(t_full=key[0], t_local=key[1],
                                            loop_reps=key[2], probe=probe)
    return _PROGRAM_CACHE[key]


def kernel(**inputs):
    x = np.asarray(inputs["x"])
    adj = np.asarray(inputs["adj_matrix"])
    nc = _get_program()
    in_maps = shard_inputs(x, adj, np.asarray(inputs["w_qkv"]),
                           np.asarray(inputs["b_qkv"]),
                           np.asarray(inputs["w_proj"]),
                           np.asarray(inputs["b_proj"]))
    res = run_bass_kernel_spmd(nc, in_maps, list(range(NCORES)))
    out = np.empty((B, T, C), dtype=np.float32)
    for core in range(NCORES):
        b, th = divmod(core, 2)
        out[b, th * TL:(th + 1) * TL, :] = res.results[core]["out"]
    return out

